# revision 19
# baseline (speedup 1.0000x reference)
"""CARAFE content-aware upsampling on 8 Trainium2 NeuronCores (Bass/Tile).

Problem: x[2,256,64,64], 1x1 compress conv (256->32), 5x5 encoder conv
(32->100), pixel-shuffle(r=2) + softmax over 25 taps, then dynamic-filter
reassembly: out[b,c,2h+r1,2w+r2] = sum_k x[b,c,h+di,w+dj] * softmax_w.

Sharding: pure data-parallel over (batch, 16-row H slices) -> 8 cores.
Each core receives its zero-padded input slice (halo rows pre-padded in
numpy) and computes a [256, 32, 128] output slice.

This implementation is tuned for the per-DMA descriptor-generation cost
(HWDGE is a serialized device at ~0.6us per DMA): everything is bf16 and
the DMA count is minimized:
  - the MAC stationaries (im2col windows) are pre-gathered in numpy and
    loaded as 2 large DMAs instead of 192 SBUF-SBUF gathers;
  - encoder channels are reordered to sub-major (o' = sub*25 + tap) so
    the softmax output lands as yM2[25, (w, sub, tb)] with a contiguous
    128-wide (sub, tb) block, letting the band-matrix scatter run as 10
    DMAs into a zeroed DRAM bounce buffer (DRAM APs have no partition
    constraints, so the (w -> +1 row, +128 col) diagonal is one stride)
    followed by 2 dense loads back into SBUF;
  - compress/encoder/softmax/MAC matmuls all run in bf16 (1 cycle/row
    vs 4 for fp32);
  - outputs are stored as 4 merged bf16 DMAs and upcast on the host.
"""

import sys

sys.path.insert(0, "/opt/trn_rl_repo")

import ml_dtypes
import numpy as np

import concourse.bacc as bacc
import concourse.tile as tile
from concourse import mybir
from concourse.ap import AP

F32 = mybir.dt.float32
BF16 = mybir.dt.bfloat16
BF16_NP = ml_dtypes.bfloat16

# geometry
B, C, H, W = 2, 256, 64, 64
RATIO, K_UP, C_MID, ENC_K = 2, 5, 32, 5
NK = RATIO * RATIO * K_UP * K_UP  # 100
NKP = 128                         # NK padded to 4 groups of 32 (o'' = sub*32 + tap)
HSLICE = 16                       # output source rows per core
ROWS = HSLICE + 4                 # with 2-row halo each side
WP = W + 4                        # padded width
PADPOS = ROWS * WP                # 1360
NCORES = 8
KDIM = 120                        # 6 rows x 20 cols window pixels per block
YHALF = 2048                      # ybig columns per ro half


def build_program(with_ebias: bool):
    nc = bacc.Bacc()
    xin_d = nc.declare_dram_parameter("xin", [128, 2 * PADPOS], BF16, isOutput=False)
    XSPLIT = 512
    xc_d = nc.declare_dram_parameter("xc", [2, KDIM, 4096], BF16, isOutput=False)
    wct_d = nc.declare_dram_parameter("wct", [128, 2 * C_MID], BF16, isOutput=False)
    wetK_d = nc.declare_dram_parameter("wetK", [128, 5 * NKP], BF16, isOutput=False)
    wet4_d = nc.declare_dram_parameter("wet4", [C_MID, 5 * NKP], BF16, isOutput=False)
    sel_d = nc.declare_dram_parameter("sel", [NKP, 4], BF16, isOutput=False)
    selt_d = nc.declare_dram_parameter("selt", [4, NKP], BF16, isOutput=False)
    # zeroed DRAM bounce buffers for the band matrix, split per row parity
    # AND per scatter-engine group so each ybig half-load only waits for its
    # own scatters: a = rows [0, 20*(ro+3)) (HWDGE scats dii 0-2),
    # b = rows [20*(ro+3), 120) (Pool scats dii 3-4)
    yza_d = [
        nc.declare_dram_parameter(f"yza{ro}", [20 * (ro + 3), YHALF], BF16,
                                  isOutput=False)
        for ro in range(2)
    ]
    yzb_d = [
        nc.declare_dram_parameter(f"yzb{ro}", [KDIM - 20 * (ro + 3), YHALF], BF16,
                                  isOutput=False)
        for ro in range(2)
    ]
    if with_ebias:
        ebias_d = nc.declare_dram_parameter("ebias", [2, NKP, 512], F32, isOutput=False)
    out_d = nc.declare_dram_parameter("out", [2, 128, 32 * 128], BF16, isOutput=True)

    with tile.TileContext(nc) as tc:
        # The byte-range race detector cannot model the diagonal scatter
        # APs (partition+free coupled strides) and reports false positives;
        # dependency generation itself is tensor-granular and conservative,
        # and every raw-AP tensor here is persistent (no slot reuse).
        tc.race_detector_enabled = False
        with (
            tc.tile_pool(name="persist", bufs=1) as pp,
            tc.tile_pool(name="psCMP", bufs=2, space="PSUM") as psCMP,
            tc.tile_pool(name="psENC", bufs=2, space="PSUM") as psENC,
            tc.tile_pool(name="psSM", bufs=1, space="PSUM") as psSM,
            tc.tile_pool(name="psMAC", bufs=3, space="PSUM") as psMAC,
        ):
            # ---- input loads: x arrives in 3 chunks so each compress
            # chunk's operand lands just before its matmuls ----
            xchunks = [(0, 512), (512, 512), (1024, PADPOS - 1024)]
            xtiles = []
            for ci, (xo, xn) in enumerate(xchunks):
                t = pp.tile([128, 2 * xn], BF16, tag=f"xin{ci}")
                xtiles.append(t)

            def load_xchunk(ci):
                xo, xn = xchunks[ci]
                nc.sync.dma_start(
                    AP(xtiles[ci].tensor, 0, [[2 * xn, 128], [xn, 2], [1, xn]]),
                    AP(xin_d, xo, [[2 * PADPOS, 128], [PADPOS, 2], [1, xn]]),
                )

            load_xchunk(0)
            wct = pp.tile([128, 2 * C_MID], BF16, tag="wct")
            nc.sync.dma_start(wct[:], wct_d[:])
            load_xchunk(1)
            load_xchunk(2)
            wetK = pp.tile([128, 5 * NKP], BF16, tag="wetK")
            nc.sync.dma_start(wetK[:], wetK_d[:])
            wet4 = pp.tile([C_MID, 5 * NKP], BF16, tag="wet4")
            nc.sync.dma_start(wet4[:], wet4_d[:])
            sel = pp.tile([NKP, 4], BF16, tag="sel")
            nc.sync.dma_start(sel[:], sel_d[:])
            selt = pp.tile([4, NKP], BF16, tag="selt")
            nc.sync.dma_start(selt[:], selt_d[:])
            xc = []
            for t in range(2):
                xct = pp.tile([KDIM, 4096], BF16, tag=f"xc{t}")
                nc.sync.dma_start(xct[:], xc_d[t])
                xc.append(xct)
            if with_ebias:
                ebias = []
                for ro in range(2):
                    t = pp.tile([NKP, 512], F32, name=f"ebias{ro}", tag=f"ebias{ro}")
                    nc.scalar.dma_start(t[:], ebias_d[ro])
                    ebias.append(t)

            # ---- compress conv: y1[32, PADPOS] bf16 ----
            y1 = pp.tile([C_MID, PADPOS], BF16, tag="y1")
            for ci, (xo, xn) in enumerate(xchunks):
                src = xtiles[ci]
                ps = psCMP.tile([C_MID, 512], F32, tag="cmp")
                for ct in range(2):
                    nc.tensor.matmul(
                        ps[:, :xn],
                        wct[:, ct * C_MID:(ct + 1) * C_MID],
                        src[:, ct * xn:ct * xn + xn],
                        start=(ct == 0), stop=(ct == 1),
                    )
                if ci % 2 == 0:
                    nc.vector.tensor_copy(y1[:, xo:xo + xn], ps[:, :xn])
                else:
                    nc.scalar.copy(y1[:, xo:xo + xn], ps[:, :xn])

            # ---- y1rep: 4 column-shifted copies of y1 packed on the
            # partition axis, so the encoder contracts (m, ej) in one K=128
            # matmul per conv row (plus a K=32 leftover for ej=4) ----
            y1rep = pp.tile([128, PADPOS], BF16, tag="y1rep")
            for ej in range(4):
                a = max(0, 2 - ej)           # dst col start
                srcs = max(0, ej - 2)        # src col start
                ncols = PADPOS - abs(ej - 2)
                dst = AP(y1rep.tensor, ej * 32 * PADPOS + a,
                         [[PADPOS, C_MID], [1, ncols]])
                srcr = AP(y1.tensor, srcs, [[PADPOS, C_MID], [1, ncols]])
                nc.vector.tensor_copy(dst, srcr)

            # ---- encoder conv for both parities (double-buffered PSUM) ----
            epss, y2es = [], []
            for ro in range(2):
                ps = psENC.tile([NKP, 512], F32, tag="enc")
                for dii in range(5):
                    rhs = AP(
                        y1rep.tensor,
                        (ro + dii) * WP + 2,
                        [[PADPOS, 128], [1, 16], [2 * WP, 8], [16, 4]],
                    )
                    nc.tensor.matmul(
                        ps[:], wetK[:, dii * NKP:(dii + 1) * NKP], rhs,
                        start=(dii == 0), stop=False,
                    )
                    rhs4 = AP(
                        y1.tensor,
                        (ro + dii) * WP + 4,
                        [[PADPOS, C_MID], [1, 16], [2 * WP, 8], [16, 4]],
                    )
                    nc.tensor.matmul(
                        ps[:], wet4[:, dii * NKP:(dii + 1) * NKP], rhs4,
                        start=False, stop=(dii == 4),
                    )
                y2e = pp.tile([NKP, 512], BF16, name=f"y2e{ro}", tag=f"y2e{ro}")
                if with_ebias:
                    nc.vector.scalar_tensor_tensor(
                        y2e[:], ps[:], 1.0, ebias[ro][:],
                        op0=mybir.AluOpType.mult, op1=mybir.AluOpType.add,
                    )
                    nc.scalar.activation(
                        y2e[:], y2e[:], mybir.ActivationFunctionType.Exp
                    )
                else:
                    nc.scalar.activation(
                        y2e[:], ps[:], mybir.ActivationFunctionType.Exp
                    )
                epss.append(ps)
                y2es.append(y2e)

            # ---- per row-parity: softmax tail + band scatter ----
            ybig = []
            for ro in range(2):
                y2e = y2es[ro]
                # tap-sums per sub (o'' = sub*32 + tap), reciprocal, broadcast
                pss = psENC.tile([4, 512], F32, tag="enc")
                nc.tensor.matmul(pss[:], sel[:], y2e[:], start=True, stop=True)
                rsum4 = pp.tile([4, 512], BF16, name=f"rsum4{ro}", tag=f"rsum4{ro}")
                with nc.allow_low_precision(
                    reason="softmax denominators are O(1); bf16 reciprocal "
                           "keeps weights within ~0.4% which is inside the "
                           "2e-2 tolerance"
                ):
                    nc.vector.reciprocal(rsum4[:], pss[:])
                psb = psSM.tile([NKP, 512], F32, tag="bcast")
                nc.tensor.matmul(psb[:], selt[:], rsum4[:], start=True, stop=True)
                # normalize in natural layout, then relayout to
                # yM2[25, (w, sub, tb)] with copies split across DVE/Act
                yMf = pp.tile([NKP, 512], BF16, name=f"yMf{ro}", tag=f"yMf{ro}")
                nc.vector.tensor_tensor(
                    yMf[:], y2e[:], psb[:], op=mybir.AluOpType.mult
                )
                yM2 = pp.tile([25, YHALF], BF16, name=f"yM2{ro}", tag=f"yM2{ro}")
                for sub in range(4):
                    dst = AP(yM2.tensor, sub * 32, [[YHALF, 25], [128, 16], [1, 32]])
                    srcr = AP(yMf.tensor, sub * 32 * 512,
                              [[512, 25], [32, 16], [1, 32]])
                    if sub % 2 == 0:
                        nc.vector.tensor_copy(dst, srcr)
                    else:
                        nc.scalar.copy(dst, srcr)
                # band scatter through the zeroed DRAM bounce buffer: the
                # (w -> +1 row, +128 col) diagonal is stride YHALF+128
                seng = nc.sync if ro == 0 else nc.scalar
                RA = 20 * (ro + 3)
                for dii in range(5):
                    src = AP(yM2.tensor, dii * 5 * YHALF, [[YHALF, 5], [1, YHALF]])
                    row0 = (ro + dii) * 20
                    if dii < 3:
                        dst = AP(yza_d[ro], row0 * YHALF,
                                 [[YHALF, 5], [YHALF + 128, 16], [1, 128]])
                        seng.dma_start(dst, src)
                    else:
                        dst = AP(yzb_d[ro], (row0 - RA) * YHALF,
                                 [[YHALF, 5], [YHALF + 128, 16], [1, 128]])
                        nc.gpsimd.dma_start(dst, src)
                yb = pp.tile([KDIM, YHALF], BF16, name=f"ybig{ro}", tag=f"ybig{ro}")
                seng.dma_start(yb[0:RA, :], yza_d[ro][:])
                seng.dma_start(yb[RA:KDIM, :], yzb_d[ro][:])
                ybig.append(yb)

            # ---- MAC: per row-pair group, dense [120]x[120] band matmuls.
            # psum tiles are per (g, ct, ro) half-banks so the whole ro=0
            # sweep (matmuls + osb copies) completes while the ro=1 band
            # matrix is still in flight.
            osbs = [[pp.tile([128, 1024], BF16, name=f"osb{i}_{r}",
                             tag=f"osb{i}_{r}") for r in range(2)]
                    for i in range(4)]
            for ro in range(2):
                for g in range(8):
                    ps = psMAC.tile([128, 512], F32, tag="mac")
                    for ct in range(2):
                        for b4 in range(4):
                            nc.tensor.matmul(
                                ps[:, ct * 256 + b4 * 64:ct * 256 + b4 * 64 + 64],
                                xc[g // 4][:, (g % 4) * 1024 + b4 * 256
                                           + ct * 128:(g % 4) * 1024 + b4 * 256
                                           + ct * 128 + 128],
                                AP(ybig[ro].tensor, g * 4 + b4,
                                   [[YHALF, KDIM], [32, 64]]),
                                start=True, stop=True,
                            )
                    # psum cols (ct, b4, w, sub) -> osb[gpair][ro] half gl
                    osb = osbs[g // 2][ro]
                    gl = g % 2
                    dst = AP(osb.tensor, gl * 256,
                             [[1024, 128], [512, 2], [1, 256]])
                    srcp = AP(ps.tensor, 0, [[512, 128], [256, 2], [1, 256]])
                    if g % 2 == 0:
                        nc.vector.tensor_copy(dst, srcp)
                    else:
                        nc.scalar.copy(dst, srcp)
                    # out_d col (per ct) = ((ro*8 + g)*4 + b4)*64 + (w, sub)
                    if gl == 1:
                        nc.sync.dma_start(
                            AP(out_d, (ro * 8 + g - 1) * 256,
                               [[4096, 128], [128 * 4096, 2], [1, 512]]),
                            AP(osb.tensor, 0,
                               [[1024, 128], [512, 2], [1, 512]]),
                        )
    nc.compile()
    return nc


_CACHE: dict[bool, object] = {}


def _get_program(with_ebias: bool):
    if with_ebias not in _CACHE:
        _CACHE[with_ebias] = build_program(with_ebias)
    return _CACHE[with_ebias]


def _prep_inputs(x, w_comp, b_comp, w_enc, b_enc):
    """Build the per-core numpy input dicts (all device tensors bf16)."""
    x = np.asarray(x, dtype=np.float32)
    w_comp = np.asarray(w_comp, dtype=np.float32)
    b_comp = np.asarray(b_comp, dtype=np.float32)
    w_enc = np.asarray(w_enc, dtype=np.float32)
    b_enc = np.asarray(b_enc, dtype=np.float32)

    # compress weights: wct[p, ct*32 + m] = w_comp[m, ct*128 + p]
    wct = np.ascontiguousarray(
        w_comp.T.reshape(2, 128, C_MID).transpose(1, 0, 2).reshape(128, 2 * C_MID)
    ).astype(BF16_NP)
    # encoder weights, channels reordered sub-major: o'' = sub*32 + tap_up,
    # conv taps (ei, ej): ej 0-3 packed on the K axis (wetK), ej=4 separate
    we = w_enc.reshape(NK, C_MID, 25)              # [o = tap*4+sub, m, etap]
    weo = we.reshape(25, 4, C_MID, 25)             # [tap_up, sub, m, etap]
    wetf = weo.transpose(2, 3, 1, 0)               # [m, etap, sub, tap_up]
    wet = np.zeros((C_MID, 5, 5, 4, 32), dtype=np.float32)
    wet[:, :, :, :, :25] = wetf.reshape(C_MID, 5, 5, 4, 25)
    # wetK[m + 32*ej, ei*128 + o''] ; wet4[m, ei*128 + o'']
    wetK = np.ascontiguousarray(
        wet[:, :, :4].transpose(2, 0, 1, 3, 4).reshape(128, 5 * NKP)
    ).astype(BF16_NP)
    wet4 = np.ascontiguousarray(
        wet[:, :, 4].reshape(C_MID, 5 * NKP)
    ).astype(BF16_NP)
    sel = np.zeros((NKP, 4), dtype=BF16_NP)
    opp = np.arange(NKP)
    real = (opp % 32) < 25
    sel[opp[real], opp[real] // 32] = 1.0
    selt = np.ascontiguousarray(sel.T)
    yza = [np.zeros((20 * (ro + 3), YHALF), dtype=BF16_NP) for ro in range(2)]
    yzb = [np.zeros((KDIM - 20 * (ro + 3), YHALF), dtype=BF16_NP) for ro in range(2)]

    with_ebias = bool(b_comp.any() or b_enc.any())

    in_maps = []
    for core in range(NCORES):
        b = core // 4
        h0 = (core % 4) * HSLICE
        xs = np.zeros((C, ROWS, WP), dtype=np.float32)
        r_lo = max(0, h0 - 2)
        r_hi = min(H, h0 + HSLICE + 2)
        xs[:, (r_lo - (h0 - 2)):(r_hi - (h0 - 2)), 2:2 + W] = x[b, :, r_lo:r_hi, :]
        xs = xs.astype(BF16_NP)
        # xin[p, ct*1360 + pos]
        xin = np.ascontiguousarray(
            xs.reshape(2, 128, PADPOS).transpose(1, 0, 2).reshape(128, 2 * PADPOS)
        )
        # im2col MAC stationaries: xc[g, r*20+wcol, b4*256 + ct*128 + c]
        xc = np.empty((8, KDIM, 1024), dtype=BF16_NP)
        for g in range(8):
            for r in range(6):
                sl = xs[:, 2 * g + r, :]            # [256, 68]
                for b4 in range(4):
                    w20 = sl[:, b4 * 16:b4 * 16 + 20]   # [256, 20]
                    xc[g, r * 20:(r + 1) * 20, b4 * 256:(b4 + 1) * 256] = w20.T
        xc2 = np.ascontiguousarray(
            xc.reshape(2, 4, KDIM, 1024).transpose(0, 2, 1, 3).reshape(2, KDIM, 4096)
        )
        m = {
            "xin": xin,
            "xc": xc2,
            "wct": wct,
            "wetK": wetK,
            "wet4": wet4,
            "sel": sel,
            "selt": selt,
            "yza0": yza[0],
            "yza1": yza[1],
            "yzb0": yzb[0],
            "yzb1": yzb[1],
        }
        if with_ebias:
            # field[o, h, w] = b_enc[o] + sum_m sum_taps_valid w_enc[o,m,tap] b_comp[m]
            wb = np.einsum("omt,m->ot", we, b_comp).reshape(NK, 5, 5)
            field = np.zeros((NK, HSLICE, W), dtype=np.float32)
            for di in range(-2, 3):
                for dj in range(-2, 3):
                    hh = np.arange(h0, h0 + HSLICE)[:, None] + di
                    ww = np.arange(W)[None, :] + dj
                    valid = ((hh >= 0) & (hh < H) & (ww >= 0) & (ww < W))
                    field += (
                        wb[:, di + 2, dj + 2][:, None, None]
                        * valid[None].astype(np.float32)
                    )
            field += b_enc[:, None, None]
            # reorder o -> o' = sub*25 + tap_up, then pos' = (w, tile, b4)
            fieldp = np.zeros((4, 32, HSLICE, W), dtype=np.float32)
            fieldp[:, :25] = field.reshape(25, 4, HSLICE, W).transpose(1, 0, 2, 3)
            f = fieldp.reshape(NKP, 8, 2, 4, 16)      # (o'', tile, ro, b4, w)
            f = np.transpose(f, (2, 0, 4, 1, 3))      # (ro, o'', w, tile, b4)
            m["ebias"] = np.ascontiguousarray(f.reshape(2, NKP, 512))
        in_maps.append(m)
    return in_maps, with_ebias


TRACE = False
LAST_RESULT = None


def kernel(x, w_comp, b_comp, w_enc, b_enc):
    global LAST_RESULT
    from concourse.bass_utils import run_bass_kernel_spmd

    in_maps, with_ebias = _prep_inputs(x, w_comp, b_comp, w_enc, b_enc)
    nc = _get_program(with_ebias)
    res = run_bass_kernel_spmd(
        nc, in_maps, core_ids=list(range(NCORES)), trace=TRACE
    )
    LAST_RESULT = res
    out = np.empty((B, C, 2 * H, 2 * W), dtype=np.float32)
    for core in range(NCORES):
        b = core // 4
        h0 = (core % 4) * HSLICE
        o = np.asarray(res.results[core]["out"]).astype(np.float32)
        o = o.reshape(2, 128, 2, 8, 4, 16, 2, 2)
        # axes: (ct, c, ro, g, b4, w, r1, r2) -> (ct, c, g, ro, r1, b4, w, r2)
        o = np.transpose(o, (0, 1, 3, 2, 6, 4, 5, 7)).reshape(2, 128, 32, 128)
        out[b, :128, 2 * h0:2 * h0 + 32, :] = o[0]
        out[b, 128:, 2 * h0:2 * h0 + 32, :] = o[1]
    return out


# revision 20
# speedup vs baseline: 1.0053x; 1.0053x over previous
"""CARAFE content-aware upsampling on 8 Trainium2 NeuronCores (Bass/Tile).

Problem: x[2,256,64,64], 1x1 compress conv (256->32), 5x5 encoder conv
(32->100), pixel-shuffle(r=2) + softmax over 25 taps, then dynamic-filter
reassembly: out[b,c,2h+r1,2w+r2] = sum_k x[b,c,h+di,w+dj] * softmax_w.

Sharding: pure data-parallel over (batch, 16-row H slices) -> 8 cores.
Each core receives its zero-padded input slice (halo rows pre-padded in
numpy) and computes a [256, 32, 128] output slice.

This implementation is tuned for the per-DMA descriptor-generation cost
(HWDGE is a serialized device at ~0.6us per DMA): everything is bf16 and
the DMA count is minimized:
  - the MAC stationaries (im2col windows) are pre-gathered in numpy and
    loaded as 2 large DMAs instead of 192 SBUF-SBUF gathers;
  - encoder channels are reordered to sub-major (o' = sub*25 + tap) so
    the softmax output lands as yM2[25, (w, sub, tb)] with a contiguous
    128-wide (sub, tb) block, letting the band-matrix scatter run as 10
    DMAs into a zeroed DRAM bounce buffer (DRAM APs have no partition
    constraints, so the (w -> +1 row, +128 col) diagonal is one stride)
    followed by 2 dense loads back into SBUF;
  - compress/encoder/softmax/MAC matmuls all run in bf16 (1 cycle/row
    vs 4 for fp32);
  - outputs are stored as 4 merged bf16 DMAs and upcast on the host.
"""

import sys

sys.path.insert(0, "/opt/trn_rl_repo")

import ml_dtypes
import numpy as np

import concourse.bacc as bacc
import concourse.tile as tile
from concourse import mybir
from concourse.ap import AP

F32 = mybir.dt.float32
BF16 = mybir.dt.bfloat16
BF16_NP = ml_dtypes.bfloat16

# geometry
B, C, H, W = 2, 256, 64, 64
RATIO, K_UP, C_MID, ENC_K = 2, 5, 32, 5
NK = RATIO * RATIO * K_UP * K_UP  # 100
NKP = 128                         # NK padded to 4 groups of 32 (o'' = sub*32 + tap)
HSLICE = 16                       # output source rows per core
ROWS = HSLICE + 4                 # with 2-row halo each side
WP = W + 4                        # padded width
PADPOS = ROWS * WP                # 1360
NCORES = 8
KDIM = 120                        # 6 rows x 20 cols window pixels per block
YHALF = 2048                      # ybig columns per ro half


def build_program(with_ebias: bool):
    nc = bacc.Bacc()
    xin_d = nc.declare_dram_parameter("xin", [128, 2 * PADPOS], BF16, isOutput=False)
    XSPLIT = 512
    xc_d = nc.declare_dram_parameter("xc", [2, KDIM, 4096], BF16, isOutput=False)
    wct_d = nc.declare_dram_parameter("wct", [128, 2 * C_MID], BF16, isOutput=False)
    wetK_d = nc.declare_dram_parameter("wetK", [128, 5 * NKP], BF16, isOutput=False)
    wet4_d = nc.declare_dram_parameter("wet4", [C_MID, 5 * NKP], BF16, isOutput=False)
    sel_d = nc.declare_dram_parameter("sel", [NKP, 4], BF16, isOutput=False)
    selt_d = nc.declare_dram_parameter("selt", [4, NKP], BF16, isOutput=False)
    # zeroed DRAM bounce buffers for the band matrix, split per row parity
    # AND per scatter-engine group so each ybig half-load only waits for its
    # own scatters: a = rows [0, 20*(ro+3)) (HWDGE scats dii 0-2),
    # b = rows [20*(ro+3), 120) (Pool scats dii 3-4)
    yza_d = [
        nc.declare_dram_parameter(f"yza{ro}", [20 * (ro + 3), YHALF], BF16,
                                  isOutput=False)
        for ro in range(2)
    ]
    yzb_d = [
        nc.declare_dram_parameter(f"yzb{ro}", [KDIM - 20 * (ro + 3), YHALF], BF16,
                                  isOutput=False)
        for ro in range(2)
    ]
    if with_ebias:
        ebias_d = nc.declare_dram_parameter("ebias", [2, NKP, 512], F32, isOutput=False)
    out_d = nc.declare_dram_parameter("out", [2, 128, 32 * 128], BF16, isOutput=True)

    with tile.TileContext(nc) as tc:
        # The byte-range race detector cannot model the diagonal scatter
        # APs (partition+free coupled strides) and reports false positives;
        # dependency generation itself is tensor-granular and conservative,
        # and every raw-AP tensor here is persistent (no slot reuse).
        tc.race_detector_enabled = False
        with (
            tc.tile_pool(name="persist", bufs=1) as pp,
            tc.tile_pool(name="psCMP", bufs=2, space="PSUM") as psCMP,
            tc.tile_pool(name="psENC", bufs=2, space="PSUM") as psENC,
            tc.tile_pool(name="psSM", bufs=1, space="PSUM") as psSM,
            tc.tile_pool(name="psMAC", bufs=3, space="PSUM") as psMAC,
        ):
            # ---- input loads: x arrives in 3 chunks so each compress
            # chunk's operand lands just before its matmuls ----
            xchunks = [(0, 512), (512, 512), (1024, PADPOS - 1024)]
            xtiles = []
            for ci, (xo, xn) in enumerate(xchunks):
                t = pp.tile([128, 2 * xn], BF16, tag=f"xin{ci}")
                xtiles.append(t)

            def load_xchunk(ci):
                xo, xn = xchunks[ci]
                nc.sync.dma_start(
                    AP(xtiles[ci].tensor, 0, [[2 * xn, 128], [xn, 2], [1, xn]]),
                    AP(xin_d, xo, [[2 * PADPOS, 128], [PADPOS, 2], [1, xn]]),
                )

            load_xchunk(0)
            wct = pp.tile([128, 2 * C_MID], BF16, tag="wct")
            nc.gpsimd.dma_start(wct[:], wct_d[:])
            load_xchunk(1)
            load_xchunk(2)
            wetK = pp.tile([128, 5 * NKP], BF16, tag="wetK")
            nc.sync.dma_start(wetK[:], wetK_d[:])
            wet4 = pp.tile([C_MID, 5 * NKP], BF16, tag="wet4")
            nc.sync.dma_start(wet4[:], wet4_d[:])
            sel = pp.tile([NKP, 4], BF16, tag="sel")
            nc.gpsimd.dma_start(sel[:], sel_d[:])
            selt = pp.tile([4, NKP], BF16, tag="selt")
            nc.gpsimd.dma_start(selt[:], selt_d[:])
            xc = []
            for t in range(2):
                xct = pp.tile([KDIM, 4096], BF16, tag=f"xc{t}")
                nc.sync.dma_start(xct[:], xc_d[t])
                xc.append(xct)
            if with_ebias:
                ebias = []
                for ro in range(2):
                    t = pp.tile([NKP, 512], F32, name=f"ebias{ro}", tag=f"ebias{ro}")
                    nc.scalar.dma_start(t[:], ebias_d[ro])
                    ebias.append(t)

            # ---- compress conv: y1[32, PADPOS] bf16 ----
            y1 = pp.tile([C_MID, PADPOS], BF16, tag="y1")
            for ci, (xo, xn) in enumerate(xchunks):
                src = xtiles[ci]
                ps = psCMP.tile([C_MID, 512], F32, tag="cmp")
                for ct in range(2):
                    nc.tensor.matmul(
                        ps[:, :xn],
                        wct[:, ct * C_MID:(ct + 1) * C_MID],
                        src[:, ct * xn:ct * xn + xn],
                        start=(ct == 0), stop=(ct == 1),
                    )
                if ci % 2 == 0:
                    nc.vector.tensor_copy(y1[:, xo:xo + xn], ps[:, :xn])
                else:
                    nc.scalar.copy(y1[:, xo:xo + xn], ps[:, :xn])

            # ---- y1rep: 4 column-shifted copies of y1 packed on the
            # partition axis, so the encoder contracts (m, ej) in one K=128
            # matmul per conv row (plus a K=32 leftover for ej=4) ----
            y1rep = pp.tile([128, PADPOS], BF16, tag="y1rep")
            for ej in range(4):
                a = max(0, 2 - ej)           # dst col start
                srcs = max(0, ej - 2)        # src col start
                ncols = PADPOS - abs(ej - 2)
                dst = AP(y1rep.tensor, ej * 32 * PADPOS + a,
                         [[PADPOS, C_MID], [1, ncols]])
                srcr = AP(y1.tensor, srcs, [[PADPOS, C_MID], [1, ncols]])
                nc.vector.tensor_copy(dst, srcr)

            # ---- encoder conv for both parities (double-buffered PSUM) ----
            epss, y2es = [], []
            for ro in range(2):
                ps = psENC.tile([NKP, 512], F32, tag="enc")
                for dii in range(5):
                    rhs = AP(
                        y1rep.tensor,
                        (ro + dii) * WP + 2,
                        [[PADPOS, 128], [1, 16], [2 * WP, 8], [16, 4]],
                    )
                    nc.tensor.matmul(
                        ps[:], wetK[:, dii * NKP:(dii + 1) * NKP], rhs,
                        start=(dii == 0), stop=False,
                    )
                    rhs4 = AP(
                        y1.tensor,
                        (ro + dii) * WP + 4,
                        [[PADPOS, C_MID], [1, 16], [2 * WP, 8], [16, 4]],
                    )
                    nc.tensor.matmul(
                        ps[:], wet4[:, dii * NKP:(dii + 1) * NKP], rhs4,
                        start=False, stop=(dii == 4),
                    )
                y2e = pp.tile([NKP, 512], BF16, name=f"y2e{ro}", tag=f"y2e{ro}")
                if with_ebias:
                    nc.vector.scalar_tensor_tensor(
                        y2e[:], ps[:], 1.0, ebias[ro][:],
                        op0=mybir.AluOpType.mult, op1=mybir.AluOpType.add,
                    )
                    nc.scalar.activation(
                        y2e[:], y2e[:], mybir.ActivationFunctionType.Exp
                    )
                else:
                    nc.scalar.activation(
                        y2e[:], ps[:], mybir.ActivationFunctionType.Exp
                    )
                epss.append(ps)
                y2es.append(y2e)

            # ---- per row-parity: softmax tail + band scatter ----
            ybig = []
            for ro in range(2):
                y2e = y2es[ro]
                # tap-sums per sub (o'' = sub*32 + tap), reciprocal, broadcast
                pss = psENC.tile([4, 512], F32, tag="enc")
                nc.tensor.matmul(pss[:], sel[:], y2e[:], start=True, stop=True)
                rsum4 = pp.tile([4, 512], BF16, name=f"rsum4{ro}", tag=f"rsum4{ro}")
                with nc.allow_low_precision(
                    reason="softmax denominators are O(1); bf16 reciprocal "
                           "keeps weights within ~0.4% which is inside the "
                           "2e-2 tolerance"
                ):
                    nc.vector.reciprocal(rsum4[:], pss[:])
                psb = psSM.tile([NKP, 512], F32, tag="bcast")
                nc.tensor.matmul(psb[:], selt[:], rsum4[:], start=True, stop=True)
                # normalize in natural layout, then relayout to
                # yM2[25, (w, sub, tb)] with copies split across DVE/Act
                yMf = pp.tile([NKP, 512], BF16, name=f"yMf{ro}", tag=f"yMf{ro}")
                nc.vector.tensor_tensor(
                    yMf[:], y2e[:], psb[:], op=mybir.AluOpType.mult
                )
                yM2 = pp.tile([25, YHALF], BF16, name=f"yM2{ro}", tag=f"yM2{ro}")
                for sub in range(4):
                    dst = AP(yM2.tensor, sub * 32, [[YHALF, 25], [128, 16], [1, 32]])
                    srcr = AP(yMf.tensor, sub * 32 * 512,
                              [[512, 25], [32, 16], [1, 32]])
                    if sub % 2 == 0:
                        nc.vector.tensor_copy(dst, srcr)
                    else:
                        nc.scalar.copy(dst, srcr)
                # band scatter through the zeroed DRAM bounce buffer: the
                # (w -> +1 row, +128 col) diagonal is stride YHALF+128
                seng = nc.sync if ro == 0 else nc.scalar
                RA = 20 * (ro + 3)
                for dii in range(5):
                    src = AP(yM2.tensor, dii * 5 * YHALF, [[YHALF, 5], [1, YHALF]])
                    row0 = (ro + dii) * 20
                    if dii < 3:
                        dst = AP(yza_d[ro], row0 * YHALF,
                                 [[YHALF, 5], [YHALF + 128, 16], [1, 128]])
                        seng.dma_start(dst, src)
                    else:
                        dst = AP(yzb_d[ro], (row0 - RA) * YHALF,
                                 [[YHALF, 5], [YHALF + 128, 16], [1, 128]])
                        nc.gpsimd.dma_start(dst, src)
                yb = pp.tile([KDIM, YHALF], BF16, name=f"ybig{ro}", tag=f"ybig{ro}")
                seng.dma_start(yb[0:RA, :], yza_d[ro][:])
                seng.dma_start(yb[RA:KDIM, :], yzb_d[ro][:])
                ybig.append(yb)

            # ---- MAC: per row-pair group, dense [120]x[120] band matmuls.
            # psum tiles are per (g, ct, ro) half-banks so the whole ro=0
            # sweep (matmuls + osb copies) completes while the ro=1 band
            # matrix is still in flight.
            osbs = [[pp.tile([128, 1024], BF16, name=f"osb{i}_{r}",
                             tag=f"osb{i}_{r}") for r in range(2)]
                    for i in range(4)]
            for ro in range(2):
                for g in range(8):
                    ps = psMAC.tile([128, 512], F32, tag="mac")
                    for ct in range(2):
                        for b4 in range(4):
                            nc.tensor.matmul(
                                ps[:, ct * 256 + b4 * 64:ct * 256 + b4 * 64 + 64],
                                xc[g // 4][:, (g % 4) * 1024 + b4 * 256
                                           + ct * 128:(g % 4) * 1024 + b4 * 256
                                           + ct * 128 + 128],
                                AP(ybig[ro].tensor, g * 4 + b4,
                                   [[YHALF, KDIM], [32, 64]]),
                                start=True, stop=True,
                            )
                    # psum cols (ct, b4, w, sub) -> osb[gpair][ro] half gl
                    osb = osbs[g // 2][ro]
                    gl = g % 2
                    dst = AP(osb.tensor, gl * 256,
                             [[1024, 128], [512, 2], [1, 256]])
                    srcp = AP(ps.tensor, 0, [[512, 128], [256, 2], [1, 256]])
                    if g % 2 == 0:
                        nc.vector.tensor_copy(dst, srcp)
                    else:
                        nc.scalar.copy(dst, srcp)
                    # out_d col (per ct) = ((ro*8 + g)*4 + b4)*64 + (w, sub)
                    if gl == 1:
                        nc.sync.dma_start(
                            AP(out_d, (ro * 8 + g - 1) * 256,
                               [[4096, 128], [128 * 4096, 2], [1, 512]]),
                            AP(osb.tensor, 0,
                               [[1024, 128], [512, 2], [1, 512]]),
                        )
    nc.compile()
    return nc


_CACHE: dict[bool, object] = {}


def _get_program(with_ebias: bool):
    if with_ebias not in _CACHE:
        _CACHE[with_ebias] = build_program(with_ebias)
    return _CACHE[with_ebias]


def _prep_inputs(x, w_comp, b_comp, w_enc, b_enc):
    """Build the per-core numpy input dicts (all device tensors bf16)."""
    x = np.asarray(x, dtype=np.float32)
    w_comp = np.asarray(w_comp, dtype=np.float32)
    b_comp = np.asarray(b_comp, dtype=np.float32)
    w_enc = np.asarray(w_enc, dtype=np.float32)
    b_enc = np.asarray(b_enc, dtype=np.float32)

    # compress weights: wct[p, ct*32 + m] = w_comp[m, ct*128 + p]
    wct = np.ascontiguousarray(
        w_comp.T.reshape(2, 128, C_MID).transpose(1, 0, 2).reshape(128, 2 * C_MID)
    ).astype(BF16_NP)
    # encoder weights, channels reordered sub-major: o'' = sub*32 + tap_up,
    # conv taps (ei, ej): ej 0-3 packed on the K axis (wetK), ej=4 separate
    we = w_enc.reshape(NK, C_MID, 25)              # [o = tap*4+sub, m, etap]
    weo = we.reshape(25, 4, C_MID, 25)             # [tap_up, sub, m, etap]
    wetf = weo.transpose(2, 3, 1, 0)               # [m, etap, sub, tap_up]
    wet = np.zeros((C_MID, 5, 5, 4, 32), dtype=np.float32)
    wet[:, :, :, :, :25] = wetf.reshape(C_MID, 5, 5, 4, 25)
    # wetK[m + 32*ej, ei*128 + o''] ; wet4[m, ei*128 + o'']
    wetK = np.ascontiguousarray(
        wet[:, :, :4].transpose(2, 0, 1, 3, 4).reshape(128, 5 * NKP)
    ).astype(BF16_NP)
    wet4 = np.ascontiguousarray(
        wet[:, :, 4].reshape(C_MID, 5 * NKP)
    ).astype(BF16_NP)
    sel = np.zeros((NKP, 4), dtype=BF16_NP)
    opp = np.arange(NKP)
    real = (opp % 32) < 25
    sel[opp[real], opp[real] // 32] = 1.0
    selt = np.ascontiguousarray(sel.T)
    yza = [np.zeros((20 * (ro + 3), YHALF), dtype=BF16_NP) for ro in range(2)]
    yzb = [np.zeros((KDIM - 20 * (ro + 3), YHALF), dtype=BF16_NP) for ro in range(2)]

    with_ebias = bool(b_comp.any() or b_enc.any())

    in_maps = []
    for core in range(NCORES):
        b = core // 4
        h0 = (core % 4) * HSLICE
        xs = np.zeros((C, ROWS, WP), dtype=np.float32)
        r_lo = max(0, h0 - 2)
        r_hi = min(H, h0 + HSLICE + 2)
        xs[:, (r_lo - (h0 - 2)):(r_hi - (h0 - 2)), 2:2 + W] = x[b, :, r_lo:r_hi, :]
        xs = xs.astype(BF16_NP)
        # xin[p, ct*1360 + pos]
        xin = np.ascontiguousarray(
            xs.reshape(2, 128, PADPOS).transpose(1, 0, 2).reshape(128, 2 * PADPOS)
        )
        # im2col MAC stationaries: xc[g, r*20+wcol, b4*256 + ct*128 + c]
        xc = np.empty((8, KDIM, 1024), dtype=BF16_NP)
        for g in range(8):
            for r in range(6):
                sl = xs[:, 2 * g + r, :]            # [256, 68]
                for b4 in range(4):
                    w20 = sl[:, b4 * 16:b4 * 16 + 20]   # [256, 20]
                    xc[g, r * 20:(r + 1) * 20, b4 * 256:(b4 + 1) * 256] = w20.T
        xc2 = np.ascontiguousarray(
            xc.reshape(2, 4, KDIM, 1024).transpose(0, 2, 1, 3).reshape(2, KDIM, 4096)
        )
        m = {
            "xin": xin,
            "xc": xc2,
            "wct": wct,
            "wetK": wetK,
            "wet4": wet4,
            "sel": sel,
            "selt": selt,
            "yza0": yza[0],
            "yza1": yza[1],
            "yzb0": yzb[0],
            "yzb1": yzb[1],
        }
        if with_ebias:
            # field[o, h, w] = b_enc[o] + sum_m sum_taps_valid w_enc[o,m,tap] b_comp[m]
            wb = np.einsum("omt,m->ot", we, b_comp).reshape(NK, 5, 5)
            field = np.zeros((NK, HSLICE, W), dtype=np.float32)
            for di in range(-2, 3):
                for dj in range(-2, 3):
                    hh = np.arange(h0, h0 + HSLICE)[:, None] + di
                    ww = np.arange(W)[None, :] + dj
                    valid = ((hh >= 0) & (hh < H) & (ww >= 0) & (ww < W))
                    field += (
                        wb[:, di + 2, dj + 2][:, None, None]
                        * valid[None].astype(np.float32)
                    )
            field += b_enc[:, None, None]
            # reorder o -> o' = sub*25 + tap_up, then pos' = (w, tile, b4)
            fieldp = np.zeros((4, 32, HSLICE, W), dtype=np.float32)
            fieldp[:, :25] = field.reshape(25, 4, HSLICE, W).transpose(1, 0, 2, 3)
            f = fieldp.reshape(NKP, 8, 2, 4, 16)      # (o'', tile, ro, b4, w)
            f = np.transpose(f, (2, 0, 4, 1, 3))      # (ro, o'', w, tile, b4)
            m["ebias"] = np.ascontiguousarray(f.reshape(2, NKP, 512))
        in_maps.append(m)
    return in_maps, with_ebias


TRACE = False
LAST_RESULT = None


def kernel(x, w_comp, b_comp, w_enc, b_enc):
    global LAST_RESULT
    from concourse.bass_utils import run_bass_kernel_spmd

    in_maps, with_ebias = _prep_inputs(x, w_comp, b_comp, w_enc, b_enc)
    nc = _get_program(with_ebias)
    res = run_bass_kernel_spmd(
        nc, in_maps, core_ids=list(range(NCORES)), trace=TRACE
    )
    LAST_RESULT = res
    out = np.empty((B, C, 2 * H, 2 * W), dtype=np.float32)
    for core in range(NCORES):
        b = core // 4
        h0 = (core % 4) * HSLICE
        o = np.asarray(res.results[core]["out"]).astype(np.float32)
        o = o.reshape(2, 128, 2, 8, 4, 16, 2, 2)
        # axes: (ct, c, ro, g, b4, w, r1, r2) -> (ct, c, g, ro, r1, b4, w, r2)
        o = np.transpose(o, (0, 1, 3, 2, 6, 4, 5, 7)).reshape(2, 128, 32, 128)
        out[b, :128, 2 * h0:2 * h0 + 32, :] = o[0]
        out[b, 128:, 2 * h0:2 * h0 + 32, :] = o[1]
    return out


# revision 22
# speedup vs baseline: 1.0303x; 1.0249x over previous
"""CARAFE content-aware upsampling on 8 Trainium2 NeuronCores (Bass/Tile).

Problem: x[2,256,64,64], 1x1 compress conv (256->32), 5x5 encoder conv
(32->100), pixel-shuffle(r=2) + softmax over 25 taps, then dynamic-filter
reassembly: out[b,c,2h+r1,2w+r2] = sum_k x[b,c,h+di,w+dj] * softmax_w.

Sharding: pure data-parallel over (batch, 16-row H slices) -> 8 cores.
Each core receives its zero-padded input slice (halo rows pre-padded in
numpy) and computes a [256, 32, 128] output slice.

This implementation is tuned for the per-DMA descriptor-generation cost
(HWDGE is a serialized device at ~0.6us per DMA): everything is bf16 and
the DMA count is minimized:
  - the MAC stationaries (im2col windows) are pre-gathered in numpy and
    loaded as 2 large DMAs instead of 192 SBUF-SBUF gathers;
  - encoder channels are reordered to sub-major (o' = sub*25 + tap) so
    the softmax output lands as yM2[25, (w, sub, tb)] with a contiguous
    128-wide (sub, tb) block, letting the band-matrix scatter run as 10
    DMAs into a zeroed DRAM bounce buffer (DRAM APs have no partition
    constraints, so the (w -> +1 row, +128 col) diagonal is one stride)
    followed by 2 dense loads back into SBUF;
  - compress/encoder/softmax/MAC matmuls all run in bf16 (1 cycle/row
    vs 4 for fp32);
  - outputs are stored as 4 merged bf16 DMAs and upcast on the host.
"""

import sys

sys.path.insert(0, "/opt/trn_rl_repo")

import ml_dtypes
import numpy as np

import concourse.bacc as bacc
import concourse.tile as tile
from concourse import mybir
from concourse.ap import AP

F32 = mybir.dt.float32
BF16 = mybir.dt.bfloat16
BF16_NP = ml_dtypes.bfloat16

# geometry
B, C, H, W = 2, 256, 64, 64
RATIO, K_UP, C_MID, ENC_K = 2, 5, 32, 5
NK = RATIO * RATIO * K_UP * K_UP  # 100
NKP = 128                         # NK padded to 4 groups of 32 (o'' = sub*32 + tap)
HSLICE = 16                       # output source rows per core
ROWS = HSLICE + 4                 # with 2-row halo each side
WP = W + 4                        # padded width
PADPOS = ROWS * WP                # 1360
NCORES = 8
KDIM = 120                        # 6 rows x 20 cols window pixels per block
YHALF = 2048                      # ybig columns per ro half


def build_program(with_ebias: bool):
    nc = bacc.Bacc()
    xin_d = nc.declare_dram_parameter("xin", [128, 2 * PADPOS], BF16, isOutput=False)
    XSPLIT = 512
    xc_d = nc.declare_dram_parameter("xc", [2, KDIM, 4096], BF16, isOutput=False)
    wct_d = nc.declare_dram_parameter("wct", [128, 2 * C_MID], BF16, isOutput=False)
    wetK_d = nc.declare_dram_parameter("wetK", [128, 5 * NKP], BF16, isOutput=False)
    wet4_d = nc.declare_dram_parameter("wet4", [C_MID, 5 * NKP], BF16, isOutput=False)
    sel_d = nc.declare_dram_parameter("sel", [NKP, 4], BF16, isOutput=False)
    selt_d = nc.declare_dram_parameter("selt", [4, NKP], BF16, isOutput=False)
    # zeroed DRAM bounce buffers for the band matrix, split per row parity
    # AND per scatter-engine group so each ybig half-load only waits for its
    # own scatters: a = rows [0, 20*(ro+3)) (HWDGE scats dii 0-2),
    # b = rows [20*(ro+3), 120) (Pool scats dii 3-4)
    yza_d = [
        nc.declare_dram_parameter(f"yza{ro}", [20 * (ro + 3), YHALF], BF16,
                                  isOutput=False)
        for ro in range(2)
    ]
    yzb_d = [
        nc.declare_dram_parameter(f"yzb{ro}", [KDIM - 20 * (ro + 3), YHALF], BF16,
                                  isOutput=False)
        for ro in range(2)
    ]
    if with_ebias:
        ebias_d = nc.declare_dram_parameter("ebias", [2, NKP, 512], F32, isOutput=False)
    out_d = nc.declare_dram_parameter("out", [2, 128, 32 * 128], BF16, isOutput=True)

    with tile.TileContext(nc) as tc:
        # The byte-range race detector cannot model the diagonal scatter
        # APs (partition+free coupled strides) and reports false positives;
        # dependency generation itself is tensor-granular and conservative,
        # and every raw-AP tensor here is persistent (no slot reuse).
        tc.race_detector_enabled = False
        with (
            tc.tile_pool(name="persist", bufs=1) as pp,
            tc.tile_pool(name="psCMP", bufs=2, space="PSUM") as psCMP,
            tc.tile_pool(name="psENC", bufs=2, space="PSUM") as psENC,
            tc.tile_pool(name="psSM", bufs=1, space="PSUM") as psSM,
            tc.tile_pool(name="psMAC", bufs=3, space="PSUM") as psMAC,
        ):
            # ---- input loads: x arrives in 3 chunks so each compress
            # chunk's operand lands just before its matmuls ----
            xchunks = [(0, 256), (256, 256), (512, 256), (768, 256),
                       (1024, PADPOS - 1024)]
            xtiles = []
            for ci, (xo, xn) in enumerate(xchunks):
                t = pp.tile([128, 2 * xn], BF16, tag=f"xin{ci}")
                xtiles.append(t)

            def load_xchunk(ci):
                xo, xn = xchunks[ci]
                nc.sync.dma_start(
                    AP(xtiles[ci].tensor, 0, [[2 * xn, 128], [xn, 2], [1, xn]]),
                    AP(xin_d, xo, [[2 * PADPOS, 128], [PADPOS, 2], [1, xn]]),
                )

            load_xchunk(0)
            wct = pp.tile([128, 2 * C_MID], BF16, tag="wct")
            nc.gpsimd.dma_start(wct[:], wct_d[:])
            for ci in range(1, len(xchunks)):
                load_xchunk(ci)
            wetK = pp.tile([128, 5 * NKP], BF16, tag="wetK")
            nc.sync.dma_start(wetK[:], wetK_d[:])
            wet4 = pp.tile([C_MID, 5 * NKP], BF16, tag="wet4")
            nc.sync.dma_start(wet4[:], wet4_d[:])
            sel = pp.tile([NKP, 4], BF16, tag="sel")
            nc.gpsimd.dma_start(sel[:], sel_d[:])
            selt = pp.tile([4, NKP], BF16, tag="selt")
            nc.gpsimd.dma_start(selt[:], selt_d[:])
            xc = []
            for t in range(2):
                xct = pp.tile([KDIM, 4096], BF16, tag=f"xc{t}")
                nc.gpsimd.dma_start(xct[:], xc_d[t])
                xc.append(xct)
            if with_ebias:
                ebias = []
                for ro in range(2):
                    t = pp.tile([NKP, 512], F32, name=f"ebias{ro}", tag=f"ebias{ro}")
                    nc.scalar.dma_start(t[:], ebias_d[ro])
                    ebias.append(t)

            # ---- compress conv: y1[32, PADPOS] bf16 ----
            y1 = pp.tile([C_MID, PADPOS], BF16, tag="y1")
            for ci, (xo, xn) in enumerate(xchunks):
                src = xtiles[ci]
                ps = psCMP.tile([C_MID, 512], F32, tag="cmp")
                for ct in range(2):
                    nc.tensor.matmul(
                        ps[:, :xn],
                        wct[:, ct * C_MID:(ct + 1) * C_MID],
                        src[:, ct * xn:ct * xn + xn],
                        start=(ct == 0), stop=(ct == 1),
                    )
                if ci % 2 == 0:
                    nc.vector.tensor_copy(y1[:, xo:xo + xn], ps[:, :xn])
                else:
                    nc.scalar.copy(y1[:, xo:xo + xn], ps[:, :xn])

            # ---- y1rep: 4 column-shifted copies of y1 packed on the
            # partition axis, so the encoder contracts (m, ej) in one K=128
            # matmul per conv row (plus a K=32 leftover for ej=4) ----
            y1rep = pp.tile([128, PADPOS], BF16, tag="y1rep")
            for ej in range(4):
                a = max(0, 2 - ej)           # dst col start
                srcs = max(0, ej - 2)        # src col start
                ncols = PADPOS - abs(ej - 2)
                dst = AP(y1rep.tensor, ej * 32 * PADPOS + a,
                         [[PADPOS, C_MID], [1, ncols]])
                srcr = AP(y1.tensor, srcs, [[PADPOS, C_MID], [1, ncols]])
                nc.vector.tensor_copy(dst, srcr)

            # ---- encoder conv for both parities (double-buffered PSUM) ----
            epss, y2es = [], []
            for ro in range(2):
                ps = psENC.tile([NKP, 512], F32, tag="enc")
                for dii in range(5):
                    rhs = AP(
                        y1rep.tensor,
                        (ro + dii) * WP + 2,
                        [[PADPOS, 128], [1, 16], [2 * WP, 8], [16, 4]],
                    )
                    nc.tensor.matmul(
                        ps[:], wetK[:, dii * NKP:(dii + 1) * NKP], rhs,
                        start=(dii == 0), stop=False,
                    )
                    rhs4 = AP(
                        y1.tensor,
                        (ro + dii) * WP + 4,
                        [[PADPOS, C_MID], [1, 16], [2 * WP, 8], [16, 4]],
                    )
                    nc.tensor.matmul(
                        ps[:], wet4[:, dii * NKP:(dii + 1) * NKP], rhs4,
                        start=False, stop=(dii == 4),
                    )
                y2e = pp.tile([NKP, 512], BF16, name=f"y2e{ro}", tag=f"y2e{ro}")
                if with_ebias:
                    nc.vector.scalar_tensor_tensor(
                        y2e[:], ps[:], 1.0, ebias[ro][:],
                        op0=mybir.AluOpType.mult, op1=mybir.AluOpType.add,
                    )
                    nc.scalar.activation(
                        y2e[:], y2e[:], mybir.ActivationFunctionType.Exp
                    )
                else:
                    nc.scalar.activation(
                        y2e[:], ps[:], mybir.ActivationFunctionType.Exp
                    )
                epss.append(ps)
                y2es.append(y2e)

            # ---- per row-parity: softmax tail + band scatter ----
            ybig = []
            for ro in range(2):
                y2e = y2es[ro]
                # tap-sums per sub (o'' = sub*32 + tap), reciprocal, broadcast
                pss = psENC.tile([4, 512], F32, tag="enc")
                nc.tensor.matmul(pss[:], sel[:], y2e[:], start=True, stop=True)
                rsum4 = pp.tile([4, 512], BF16, name=f"rsum4{ro}", tag=f"rsum4{ro}")
                with nc.allow_low_precision(
                    reason="softmax denominators are O(1); bf16 reciprocal "
                           "keeps weights within ~0.4% which is inside the "
                           "2e-2 tolerance"
                ):
                    nc.vector.reciprocal(rsum4[:], pss[:])
                psb = psSM.tile([NKP, 512], F32, tag="bcast")
                nc.tensor.matmul(psb[:], selt[:], rsum4[:], start=True, stop=True)
                # normalize in natural layout, then relayout to
                # yM2[25, (w, sub, tb)] with copies split across DVE/Act
                yMf = pp.tile([NKP, 512], BF16, name=f"yMf{ro}", tag=f"yMf{ro}")
                nc.vector.tensor_tensor(
                    yMf[:], y2e[:], psb[:], op=mybir.AluOpType.mult
                )
                yM2 = pp.tile([25, YHALF], BF16, name=f"yM2{ro}", tag=f"yM2{ro}")
                for sub in range(4):
                    dst = AP(yM2.tensor, sub * 32, [[YHALF, 25], [128, 16], [1, 32]])
                    srcr = AP(yMf.tensor, sub * 32 * 512,
                              [[512, 25], [32, 16], [1, 32]])
                    if sub % 2 == 0:
                        nc.vector.tensor_copy(dst, srcr)
                    else:
                        nc.scalar.copy(dst, srcr)
                # band scatter through the zeroed DRAM bounce buffer: the
                # (w -> +1 row, +128 col) diagonal is stride YHALF+128
                seng = nc.sync if ro == 0 else nc.scalar
                RA = 20 * (ro + 3)
                for dii in range(5):
                    src = AP(yM2.tensor, dii * 5 * YHALF, [[YHALF, 5], [1, YHALF]])
                    row0 = (ro + dii) * 20
                    if dii < 3:
                        dst = AP(yza_d[ro], row0 * YHALF,
                                 [[YHALF, 5], [YHALF + 128, 16], [1, 128]])
                        seng.dma_start(dst, src)
                    else:
                        dst = AP(yzb_d[ro], (row0 - RA) * YHALF,
                                 [[YHALF, 5], [YHALF + 128, 16], [1, 128]])
                        nc.gpsimd.dma_start(dst, src)
                yb = pp.tile([KDIM, YHALF], BF16, name=f"ybig{ro}", tag=f"ybig{ro}")
                seng.dma_start(yb[0:RA, :], yza_d[ro][:])
                seng.dma_start(yb[RA:KDIM, :], yzb_d[ro][:])
                ybig.append(yb)

            # ---- MAC: per row-pair group, dense [120]x[120] band matmuls.
            # psum tiles are per (g, ct, ro) half-banks so the whole ro=0
            # sweep (matmuls + osb copies) completes while the ro=1 band
            # matrix is still in flight.
            osbs = [[pp.tile([128, 1024], BF16, name=f"osb{i}_{r}",
                             tag=f"osb{i}_{r}") for r in range(2)]
                    for i in range(4)]
            for ro in range(2):
                for g in range(8):
                    ps = psMAC.tile([128, 512], F32, tag="mac")
                    for ct in range(2):
                        for b4 in range(4):
                            nc.tensor.matmul(
                                ps[:, ct * 256 + b4 * 64:ct * 256 + b4 * 64 + 64],
                                xc[g // 4][:, (g % 4) * 1024 + b4 * 256
                                           + ct * 128:(g % 4) * 1024 + b4 * 256
                                           + ct * 128 + 128],
                                AP(ybig[ro].tensor, g * 4 + b4,
                                   [[YHALF, KDIM], [32, 64]]),
                                start=True, stop=True,
                            )
                    # psum cols (ct, b4, w, sub) -> osb[gpair][ro] half gl
                    osb = osbs[g // 2][ro]
                    gl = g % 2
                    dst = AP(osb.tensor, gl * 256,
                             [[1024, 128], [512, 2], [1, 256]])
                    srcp = AP(ps.tensor, 0, [[512, 128], [256, 2], [1, 256]])
                    if g % 2 == 0:
                        nc.vector.tensor_copy(dst, srcp)
                    else:
                        nc.scalar.copy(dst, srcp)
                    # out_d col (per ct) = ((ro*8 + g)*4 + b4)*64 + (w, sub)
                    if gl == 1:
                        nc.sync.dma_start(
                            AP(out_d, (ro * 8 + g - 1) * 256,
                               [[4096, 128], [128 * 4096, 2], [1, 512]]),
                            AP(osb.tensor, 0,
                               [[1024, 128], [512, 2], [1, 512]]),
                        )
    nc.compile()
    return nc


_CACHE: dict[bool, object] = {}


def _get_program(with_ebias: bool):
    if with_ebias not in _CACHE:
        _CACHE[with_ebias] = build_program(with_ebias)
    return _CACHE[with_ebias]


def _prep_inputs(x, w_comp, b_comp, w_enc, b_enc):
    """Build the per-core numpy input dicts (all device tensors bf16)."""
    x = np.asarray(x, dtype=np.float32)
    w_comp = np.asarray(w_comp, dtype=np.float32)
    b_comp = np.asarray(b_comp, dtype=np.float32)
    w_enc = np.asarray(w_enc, dtype=np.float32)
    b_enc = np.asarray(b_enc, dtype=np.float32)

    # compress weights: wct[p, ct*32 + m] = w_comp[m, ct*128 + p]
    wct = np.ascontiguousarray(
        w_comp.T.reshape(2, 128, C_MID).transpose(1, 0, 2).reshape(128, 2 * C_MID)
    ).astype(BF16_NP)
    # encoder weights, channels reordered sub-major: o'' = sub*32 + tap_up,
    # conv taps (ei, ej): ej 0-3 packed on the K axis (wetK), ej=4 separate
    we = w_enc.reshape(NK, C_MID, 25)              # [o = tap*4+sub, m, etap]
    weo = we.reshape(25, 4, C_MID, 25)             # [tap_up, sub, m, etap]
    wetf = weo.transpose(2, 3, 1, 0)               # [m, etap, sub, tap_up]
    wet = np.zeros((C_MID, 5, 5, 4, 32), dtype=np.float32)
    wet[:, :, :, :, :25] = wetf.reshape(C_MID, 5, 5, 4, 25)
    # wetK[m + 32*ej, ei*128 + o''] ; wet4[m, ei*128 + o'']
    wetK = np.ascontiguousarray(
        wet[:, :, :4].transpose(2, 0, 1, 3, 4).reshape(128, 5 * NKP)
    ).astype(BF16_NP)
    wet4 = np.ascontiguousarray(
        wet[:, :, 4].reshape(C_MID, 5 * NKP)
    ).astype(BF16_NP)
    sel = np.zeros((NKP, 4), dtype=BF16_NP)
    opp = np.arange(NKP)
    real = (opp % 32) < 25
    sel[opp[real], opp[real] // 32] = 1.0
    selt = np.ascontiguousarray(sel.T)
    yza = [np.zeros((20 * (ro + 3), YHALF), dtype=BF16_NP) for ro in range(2)]
    yzb = [np.zeros((KDIM - 20 * (ro + 3), YHALF), dtype=BF16_NP) for ro in range(2)]

    with_ebias = bool(b_comp.any() or b_enc.any())

    in_maps = []
    for core in range(NCORES):
        b = core // 4
        h0 = (core % 4) * HSLICE
        xs = np.zeros((C, ROWS, WP), dtype=np.float32)
        r_lo = max(0, h0 - 2)
        r_hi = min(H, h0 + HSLICE + 2)
        xs[:, (r_lo - (h0 - 2)):(r_hi - (h0 - 2)), 2:2 + W] = x[b, :, r_lo:r_hi, :]
        xs = xs.astype(BF16_NP)
        # xin[p, ct*1360 + pos]
        xin = np.ascontiguousarray(
            xs.reshape(2, 128, PADPOS).transpose(1, 0, 2).reshape(128, 2 * PADPOS)
        )
        # im2col MAC stationaries: xc[g, r*20+wcol, b4*256 + ct*128 + c]
        xc = np.empty((8, KDIM, 1024), dtype=BF16_NP)
        for g in range(8):
            for r in range(6):
                sl = xs[:, 2 * g + r, :]            # [256, 68]
                for b4 in range(4):
                    w20 = sl[:, b4 * 16:b4 * 16 + 20]   # [256, 20]
                    xc[g, r * 20:(r + 1) * 20, b4 * 256:(b4 + 1) * 256] = w20.T
        xc2 = np.ascontiguousarray(
            xc.reshape(2, 4, KDIM, 1024).transpose(0, 2, 1, 3).reshape(2, KDIM, 4096)
        )
        m = {
            "xin": xin,
            "xc": xc2,
            "wct": wct,
            "wetK": wetK,
            "wet4": wet4,
            "sel": sel,
            "selt": selt,
            "yza0": yza[0],
            "yza1": yza[1],
            "yzb0": yzb[0],
            "yzb1": yzb[1],
        }
        if with_ebias:
            # field[o, h, w] = b_enc[o] + sum_m sum_taps_valid w_enc[o,m,tap] b_comp[m]
            wb = np.einsum("omt,m->ot", we, b_comp).reshape(NK, 5, 5)
            field = np.zeros((NK, HSLICE, W), dtype=np.float32)
            for di in range(-2, 3):
                for dj in range(-2, 3):
                    hh = np.arange(h0, h0 + HSLICE)[:, None] + di
                    ww = np.arange(W)[None, :] + dj
                    valid = ((hh >= 0) & (hh < H) & (ww >= 0) & (ww < W))
                    field += (
                        wb[:, di + 2, dj + 2][:, None, None]
                        * valid[None].astype(np.float32)
                    )
            field += b_enc[:, None, None]
            # reorder o -> o' = sub*25 + tap_up, then pos' = (w, tile, b4)
            fieldp = np.zeros((4, 32, HSLICE, W), dtype=np.float32)
            fieldp[:, :25] = field.reshape(25, 4, HSLICE, W).transpose(1, 0, 2, 3)
            f = fieldp.reshape(NKP, 8, 2, 4, 16)      # (o'', tile, ro, b4, w)
            f = np.transpose(f, (2, 0, 4, 1, 3))      # (ro, o'', w, tile, b4)
            m["ebias"] = np.ascontiguousarray(f.reshape(2, NKP, 512))
        in_maps.append(m)
    return in_maps, with_ebias


TRACE = False
LAST_RESULT = None


def kernel(x, w_comp, b_comp, w_enc, b_enc):
    global LAST_RESULT
    from concourse.bass_utils import run_bass_kernel_spmd

    in_maps, with_ebias = _prep_inputs(x, w_comp, b_comp, w_enc, b_enc)
    nc = _get_program(with_ebias)
    res = run_bass_kernel_spmd(
        nc, in_maps, core_ids=list(range(NCORES)), trace=TRACE
    )
    LAST_RESULT = res
    out = np.empty((B, C, 2 * H, 2 * W), dtype=np.float32)
    for core in range(NCORES):
        b = core // 4
        h0 = (core % 4) * HSLICE
        o = np.asarray(res.results[core]["out"]).astype(np.float32)
        o = o.reshape(2, 128, 2, 8, 4, 16, 2, 2)
        # axes: (ct, c, ro, g, b4, w, r1, r2) -> (ct, c, g, ro, r1, b4, w, r2)
        o = np.transpose(o, (0, 1, 3, 2, 6, 4, 5, 7)).reshape(2, 128, 32, 128)
        out[b, :128, 2 * h0:2 * h0 + 32, :] = o[0]
        out[b, 128:, 2 * h0:2 * h0 + 32, :] = o[1]
    return out
